# revision 29
# baseline (speedup 1.0000x reference)
"""Trainium2 Bass kernel for nn_DEQTransformerLM (Transformer-XL style DEQ layer).

Sharding: local-window attention (window 256, qlen 512, mlen 512) makes the
layer fully token-parallel: 8 cores = 4 batches x 2 query-halves of 256
queries each. Each core runs qkv projections, relative-position attention,
the output projection + layernorm, and the FF block end-to-end for its 256
tokens. No collectives; weights replicated (streamed from HBM as fp16).

rel_shift via DRAM scratch: BD_raw rows [q, 256] written as 385-wide padded
rows (pad = -30000), read back as contiguous [q, 384] rows (stride 384),
which lands BD_raw[i, r] at score column i+1+r and -30000 (additive mask)
elsewhere - mask and shift in one DMA.

Attention softmax pipeline (per 128-query x 384-key quarter):
  AC matmul -> PSUM; BD band added into the same PSUM tile via an
  identity-matmul accumulate; one scalar-engine Exp with accum_out produces
  fp16 probs AND the per-query denominator; [128,1] reciprocal + per-partition
  normalize; fp16 PE transposes; AV accumulated into a shared [128,128] tile.

Phases are arranged so the two hardware DMA queues (sync/scalar, ~120GB/s
each) stay balanced against tensor work; the gpsimd software-DGE queue
carries the BD scratch writes. Q/R projections + BD + attention run as one
software pipeline so the tensor engine never waits on the scratch roundtrip
or the softmax chain.
"""

import numpy as np

import concourse.bacc as bacc
import concourse.mybir as mybir
import concourse.tile as tile
from concourse.bass_utils import run_bass_kernel_spmd
from concourse.masks import make_identity

FP16 = mybir.dt.float16
FP32 = mybir.dt.float32
AF = mybir.ActivationFunctionType
ALU = mybir.AluOpType

D = 1024          # d_model
NH = 16           # heads
DH = 64           # d_head
QC = 256          # queries per core
KW = 512          # key window per core
RW = 256          # relative positions used (cols 768:1024 of r_head_k)
DI = 4096         # d_inner
NC_ = D // 128    # 8 d_model chunks
NI = DI // 128    # 32 d_inner chunks
SCR_ROW = 385     # padded scratch row width
SCR_HEAD = 2 * 128 * SCR_ROW  # scratch block per head
NEG = -30000.0    # additive mask value (fp16-safe)
BW = 384          # in-band score width per 128-query chunk

_CACHE = {}


def build_nc():
    nc = bacc.Bacc("TRN2", target_bir_lowering=False, debug=False)
    P = nc.declare_dram_parameter

    # weights pre-blocked on host as [mc, 128, kc*128] (lhsT column blocks,
    # partition-major) so each block load is one contiguous 2KB-row DMA
    wq_b = P("wq_b", [NC_, 128, D], FP16, isOutput=False)
    wk_b = P("wk_b", [NC_, 128, D], FP16, isOutput=False)
    rw_b = P("rw_b", [NC_, 128, D], FP16, isOutput=False)
    ow_b = P("ow_b", [NC_, 128, D], FP16, isOutput=False)
    ff1w_b = P("ff1w_b", [NI, 128, D], FP16, isOutput=False)
    ff2w_b = P("ff2w_b", [NC_, 2, 128, DI // 2], FP16, isOutput=False)
    wv_p = P("wv_p", [128, NC_, D], FP16, isOutput=False)   # Wv.T row-chunks

    # activations pre-arranged to [128, chunk, cols]
    xq = P("xq", [128, NC_, QC], FP16, isOutput=False)
    xkv = P("xkv", [128, NC_, KW], FP16, isOutput=False)
    uq = P("uq", [128, NC_, QC], FP16, isOutput=False)
    uk = P("uk", [128, NC_, KW], FP16, isOutput=False)
    uvt = P("uvt", [128, 4, D], FP16, isOutput=False)
    pos = P("pos", [128, NC_, RW], FP16, isOutput=False)
    bwc = P("bwc", [128, NC_], FP32, isOutput=False)
    brc = P("brc", [128, NC_], FP32, isOutput=False)
    ob = P("ob", [128, NC_], FP32, isOutput=False)
    f1b = P("f1b", [128, NI], FP32, isOutput=False)
    f2b = P("f2b", [128, NC_], FP32, isOutput=False)

    out = P("out", [D, QC], FP16, isOutput=True)

    scratch = nc.dram_tensor("scratch", [NH * SCR_HEAD], FP16)

    with tile.TileContext(nc) as tc:
        with (
            tc.tile_pool(name="single", bufs=1) as single,
            tc.tile_pool(name="wblk", bufs=6) as wblk_pool,
            tc.tile_pool(name="ff1p", bufs=10) as ff1_pool,
            tc.tile_pool(name="ff2p", bufs=4) as ff2_pool,
            tc.tile_pool(name="bdm", bufs=12) as bdm_pool,
            tc.tile_pool(name="probs", bufs=3) as probs_pool,
            tc.tile_pool(name="pn", bufs=3) as pn_pool,
            tc.tile_pool(name="den", bufs=8) as den_pool,
            tc.tile_pool(name="pt", bufs=3) as pt_pool,
            tc.tile_pool(name="rows", bufs=2) as rows_pool,
            tc.tile_pool(name="outp", bufs=2) as out_pool,
            tc.tile_pool(name="mm_ps", bufs=3, space="PSUM") as mm_ps,
            tc.tile_pool(name="bt_ps", bufs=2, space="PSUM") as bt_ps,
            tc.tile_pool(name="av_ps", bufs=2, space="PSUM") as av_ps_pool,
            tc.tile_pool(name="red_ps", bufs=1, space="PSUM") as red_ps,
        ):
            # ---- loads, ordered by first use. sync + scalar are the two
            # hardware DMA queues; gpsimd (software DGE) carries the BD
            # scratch writes + uvt so the hardware queues stay balanced.
            def load3(name, src, n, w, engine, dtype=FP16, splits=1):
                t = single.tile([128, n, w], dtype, tag=name)
                step = n // splits
                for s in range(splits):
                    engine.dma_start(out=t[:, s * step:(s + 1) * step, :],
                                     in_=src[:][:, s * step:(s + 1) * step, :])
                return t

            xkv_sb = load3("xkv", xkv, NC_, KW, nc.scalar, splits=4)
            uk_sb = load3("uk", uk, NC_, KW, nc.scalar)
            xq_sb = load3("xq", xq, NC_, QC, nc.scalar)
            uq_sb = load3("uq", uq, NC_, QC, nc.scalar)
            pos_sb = load3("pos", pos, NC_, RW, nc.scalar)
            uvt_sb = load3("uvt", uvt, 4, D, nc.gpsimd)

            def loadsmall(name, src, w):
                t = single.tile([128, w], FP32, tag=name)
                nc.scalar.dma_start(out=t, in_=src[:])
                return t

            bwc_sb = loadsmall("bwc", bwc, NC_)
            brc_sb = loadsmall("brc", brc, NC_)
            ob_sb = loadsmall("ob", ob, NC_)
            f1b_sb = loadsmall("f1b", f1b, NI)
            f2b_sb = loadsmall("f2b", f2b, NC_)

            ident16 = single.tile([128, 128], FP16, tag="ident16")
            make_identity(nc, ident16)
            ones_k = single.tile([128, 1], FP16, tag="ones_k")
            nc.vector.memset(ones_k, 1.0)
            ones_m = single.tile([1, 128], FP16, tag="ones_m")
            nc.vector.memset(ones_m, 1.0)
            eps_sb = single.tile([1, 1], FP32, tag="eps")
            nc.vector.memset(eps_sb, 1e-5)
            negc_sb = single.tile([128, 1], FP32, tag="negc")
            nc.vector.memset(negc_sb, -12.5)

            pads = single.tile([128, 8, SCR_ROW], FP16, tag="pads")
            nc.gpsimd.memset(pads, NEG)

            k_sb = single.tile([128, NC_, KW], FP16, tag="k_sb")
            vT_sb = single.tile([128, 4, D], FP16, tag="vT_sb")
            rwq_sb = single.tile([128, NC_, QC], FP16, tag="rwq")
            rrq_sb = single.tile([128, NC_, QC], FP16, tag="rrq")
            r_sb = single.tile([128, NC_, RW], FP16, tag="r_sb")
            attn_sb = single.tile([128, NC_, QC], FP16, tag="attn")
            x_sb = single.tile([128, NC_, QC], FP16, tag="x_sb")
            h_sb = single.tile([128, NI, QC], FP16, tag="h_sb")

            def wload(param, mc, engine=None):
                t = wblk_pool.tile([128, D], FP16, tag="wblk")
                (engine or nc.sync).dma_start(out=t, in_=param[:][mc])
                return t

            scr = scratch[:]

            # ---------------- phase A1: K projection ----------
            for mc in range(NC_):
                wt = wload(wk_b, mc)
                ps = mm_ps.tile([128, KW], FP32, tag="mm")
                for kc in range(NC_):
                    nc.tensor.matmul(ps, wt[:, 128 * kc:128 * (kc + 1)],
                                     xkv_sb[:, kc, :],
                                     start=(kc == 0), stop=(kc == NC_ - 1))
                nc.vector.tensor_tensor(out=k_sb[:, mc, :], in0=ps,
                                        in1=uk_sb[:, mc, :], op=ALU.add)

            # wv streams on sync during A1; V proj runs as phase A2
            wv_sb = load3("wv", wv_p, NC_, D, nc.sync)

            # ---------------- phase A2: V projection ----------
            for jb in range(4):
                for dh in range(2):
                    ps = mm_ps.tile([128, 512], FP32, tag="mm")
                    for kc in range(NC_):
                        nc.tensor.matmul(
                            ps,
                            xkv_sb[:, kc, 128 * jb:128 * (jb + 1)],
                            wv_sb[:, kc, 512 * dh:512 * (dh + 1)],
                            start=(kc == 0), stop=(kc == NC_ - 1))
                    nc.vector.tensor_tensor(
                        out=vT_sb[:, jb, 512 * dh:512 * (dh + 1)], in0=ps,
                        in1=uvt_sb[:, jb, 512 * dh:512 * (dh + 1)], op=ALU.add)

            # ------- phase B: Q/R proj + BD + attention, one pipeline -------
            def q_proj(mc):
                wt = wload(wq_b, mc, nc.sync)
                ps = mm_ps.tile([128, QC], FP32, tag="mm")
                for kc in range(NC_):
                    nc.tensor.matmul(ps, wt[:, 128 * kc:128 * (kc + 1)],
                                     xq_sb[:, kc, :],
                                     start=(kc == 0), stop=(kc == NC_ - 1))
                nc.vector.scalar_tensor_tensor(
                    out=rwq_sb[:, mc, :], in0=ps, scalar=bwc_sb[:, mc:mc + 1],
                    in1=uq_sb[:, mc, :], op0=ALU.add, op1=ALU.add)
                nc.vector.scalar_tensor_tensor(
                    out=rrq_sb[:, mc, :], in0=ps, scalar=brc_sb[:, mc:mc + 1],
                    in1=uq_sb[:, mc, :], op0=ALU.add, op1=ALU.add)

            def r_proj(mc):
                wt = wload(rw_b, mc, nc.scalar)
                ps = mm_ps.tile([128, RW], FP32, tag="mm")
                for kc in range(NC_):
                    nc.tensor.matmul(ps, wt[:, 128 * kc:128 * (kc + 1)],
                                     pos_sb[:, kc, :],
                                     start=(kc == 0), stop=(kc == NC_ - 1))
                nc.scalar.copy(r_sb[:, mc, :], ps)

            def bd_half(pi, c):
                # BD matmuls + padded scratch writes, both heads of the pair
                for sub in range(2):
                    h = 2 * pi + sub
                    hsl = slice(64 * sub, 64 * sub + 64)
                    bdp = bt_ps.tile([128, RW], FP32, tag="bt")
                    nc.tensor.matmul(bdp,
                                     rrq_sb[hsl, pi, 128 * c:128 * (c + 1)],
                                     r_sb[hsl, pi, :], start=True, stop=True)
                    slot = (pi % 2) * 4 + 2 * c + sub
                    if sub == 0:
                        nc.vector.tensor_copy(pads[:, slot, 1:257], bdp)
                    else:
                        nc.scalar.copy(pads[:, slot, 1:257], bdp)
                    wbase = h * SCR_HEAD + SCR_ROW * 128 * c
                    nc.scalar.dma_start(
                        out=scr[wbase:wbase + 128 * SCR_ROW].rearrange(
                            "(p f) -> p f", f=SCR_ROW),
                        in_=pads[:, slot, :])

            def issue_bdm(pi):
                tiles = []
                for c in range(2):
                    for sub in range(2):
                        h = 2 * pi + sub
                        rbase = h * SCR_HEAD + SCR_ROW * 128 * c
                        t = bdm_pool.tile([128, BW], FP16, tag="bdm")
                        nc.gpsimd.dma_start(
                            out=t,
                            in_=scr[rbase:rbase + 128 * BW].rearrange(
                                "(p f) -> p f", f=BW))
                        tiles.append(t)
                return tiles

            # Attention: flat 32-quarter software pipeline (quarter = one
            # 128-query chunk x one head). stage1 = score matmuls + exp +
            # normalize; stage2a = fp16 transposes; stage2b = AV. stage2 lags
            # stage1 by two quarters so the in-order tensor queue never waits
            # on the exp/normalize chain.
            def s1(pi, c, sub, bdm):
                hsl = slice(64 * sub, 64 * sub + 64)
                acp = mm_ps.tile([128, BW], FP32, tag="mm")
                nc.tensor.matmul(
                    acp, rwq_sb[hsl, pi, 128 * c:128 * (c + 1)],
                    k_sb[hsl, pi, 128 * c:128 * c + BW],
                    start=True, stop=False)
                nc.tensor.matmul(acp, ident16, bdm, start=False, stop=True)
                probs = probs_pool.tile([128, BW], FP16, tag="probs")
                den = den_pool.tile([128, 1], FP32, tag="den")
                nc.scalar.activation(out=probs, in_=acp, func=AF.Exp,
                                     scale=0.125, bias=negc_sb, accum_out=den)
                rcp = den_pool.tile([128, 1], FP32, tag="rcp")
                nc.vector.reciprocal(out=rcp, in_=den)
                pn = pn_pool.tile([128, BW], FP16, tag="pn")
                nc.vector.tensor_scalar_mul(pn, probs, rcp)
                return pn

            def s2a(qi, pn):
                tp = bt_ps.tile([128, BW], FP16, tag="bt")
                for kbp in range(3):
                    nc.tensor.transpose(tp[:, 128 * kbp:128 * (kbp + 1)],
                                        pn[:, 128 * kbp:128 * (kbp + 1)],
                                        ident16)
                ptc = pt_pool.tile([128, BW], FP16, tag="pt")
                if qi % 2 == 0:
                    nc.scalar.copy(ptc, tp)
                else:
                    nc.vector.tensor_copy(ptc, tp)
                return ptc

            av_tiles = {}

            def s2b(pi, c, sub, ptc):
                h = 2 * pi + sub
                if sub == 0:
                    av = av_ps_pool.tile([128, 128], FP32, tag="av")
                    av_tiles[(pi, c)] = av
                av = av_tiles[(pi, c)]
                for kbp in range(3):
                    nc.tensor.matmul(av[64 * sub:64 * sub + 64, :],
                                     vT_sb[:, c + kbp, 64 * h:64 * h + 64],
                                     ptc[:, 128 * kbp:128 * (kbp + 1)],
                                     start=(kbp == 0), stop=(kbp == 2))
                if sub == 1:
                    nc.scalar.copy(
                        attn_sb[:, pi, 128 * c:128 * (c + 1)],
                        av_tiles.pop((pi, c)))

            # pipeline: per mc emit Q/R/BD for chunk mc and the four attention
            # quarters of pair mc-2 (stage1), with stage2 lagging 2 quarters.
            quarters = [(pi, c, sub)
                        for pi in range(NH // 2)
                        for c in range(2) for sub in range(2)]
            NQ = len(quarters)
            bd_tiles, pns, pts = {}, {}, {}
            t = 0

            def qstep():
                nonlocal t
                if t < NQ:
                    pi, c, sub = quarters[t]
                    pns[t] = s1(pi, c, sub, bd_tiles[pi][2 * c + sub])
                if 0 <= t - 1 < NQ:
                    pts[t - 1] = s2a(t - 1, pns.pop(t - 1))
                if 0 <= t - 2 < NQ:
                    pi2, c2, sub2 = quarters[t - 2]
                    s2b(pi2, c2, sub2, pts.pop(t - 2))
                t += 1

            for mc in range(NC_):
                q_proj(mc)
                if mc >= 1:
                    bd_half(mc - 1, 0)
                r_proj(mc)
                if mc >= 1:
                    bd_half(mc - 1, 1)
                    bd_tiles[mc - 1] = issue_bdm(mc - 1)
                for _ in range(4):
                    if mc >= 2:
                        qstep()
            bd_half(NC_ - 1, 0)
            bd_half(NC_ - 1, 1)
            bd_tiles[NC_ - 1] = issue_bdm(NC_ - 1)
            while t < NQ + 2:
                qstep()

            # ---------------- O projection + LN1 ----------------
            # LN stats reduce matmuls interleave with the projection loop so
            # the post-loop serial chain is as short as possible.
            stack1 = single.tile([128, NC_, 512], FP16, tag="stack")
            red1 = red_ps.tile([1, 512], FP32, tag="red")
            for mc in range(NC_):
                wt = wload(ow_b, mc, nc.sync if mc % 2 == 0 else nc.scalar)
                ps = mm_ps.tile([128, QC], FP32, tag="mm")
                for kc in range(NC_):
                    nc.tensor.matmul(ps, wt[:, 128 * kc:128 * (kc + 1)],
                                     attn_sb[:, kc, :],
                                     start=(kc == 0), stop=(kc == NC_ - 1))
                nc.vector.scalar_tensor_tensor(
                    out=stack1[:, mc, 0:256], in0=ps, scalar=ob_sb[:, mc:mc + 1],
                    in1=xq_sb[:, mc, :], op0=ALU.add, op1=ALU.add)
                nc.vector.tensor_tensor(
                    out=stack1[:, mc, 256:512], in0=stack1[:, mc, 0:256],
                    in1=stack1[:, mc, 0:256], op=ALU.mult)
                if mc >= 2:
                    nc.tensor.matmul(red1, ones_k, stack1[:, mc - 2, :],
                                     start=(mc == 2), stop=False)
            for mc in (NC_ - 2, NC_ - 1):
                nc.tensor.matmul(red1, ones_k, stack1[:, mc, :],
                                 start=False, stop=(mc == NC_ - 1))

            def layernorm(stack, red, xout):
                mean = rows_pool.tile([1, 256], FP32, tag="mean")
                nc.vector.tensor_scalar_mul(mean, red[0:1, 0:256], 1.0 / D)
                msq = rows_pool.tile([1, 256], FP32, tag="msq")
                nc.vector.tensor_tensor(out=msq, in0=mean, in1=mean,
                                        op=ALU.mult)
                var = rows_pool.tile([1, 256], FP32, tag="var")
                nc.vector.scalar_tensor_tensor(
                    out=var, in0=red[0:1, 256:512], scalar=1.0 / D, in1=msq,
                    op0=ALU.mult, op1=ALU.subtract)
                nc.scalar.activation(out=var, in_=var, func=AF.Sqrt,
                                     bias=eps_sb, scale=1.0)
                rstd = rows_pool.tile([1, 256], FP32, tag="rstd")
                nc.vector.reciprocal_approx_fast(out=rstd, in_=var)
                rw_row = rows_pool.tile([1, 512], FP16, tag="rw_row")
                nc.vector.tensor_copy(rw_row[0:1, 0:256], rstd)
                nc.vector.tensor_tensor(out=rw_row[0:1, 256:512],
                                        in0=mean, in1=rstd, op=ALU.mult)
                bc = mm_ps.tile([128, 512], FP32, tag="mm")
                nc.tensor.matmul(bc, ones_m, rw_row, start=True, stop=True)
                bc_sb = rows_pool.tile([128, 512], FP16, tag="bc_sb")
                nc.vector.tensor_copy(bc_sb, bc)
                for mc in range(NC_):
                    xo = xout(mc)
                    eng = nc.vector if mc % 2 == 0 else nc.gpsimd
                    eng.tensor_tensor(out=xo, in0=stack[:, mc, 0:256],
                                      in1=bc_sb[:, 0:256], op=ALU.mult)
                    eng.tensor_tensor(out=xo, in0=xo,
                                      in1=bc_sb[:, 256:512], op=ALU.subtract)

            layernorm(stack1, red1, lambda mc: x_sb[:, mc, :])

            # ---------------- FF1 ----------------
            for mc in range(NI):
                wt = ff1_pool.tile([128, D], FP16, tag="ff1")
                (nc.sync if mc % 2 == 0 else nc.scalar).dma_start(
                    out=wt, in_=ff1w_b[:][mc])
                ps = mm_ps.tile([128, QC], FP32, tag="mm")
                for kc in range(NC_):
                    nc.tensor.matmul(ps, wt[:, 128 * kc:128 * (kc + 1)],
                                     x_sb[:, kc, :],
                                     start=(kc == 0), stop=(kc == NC_ - 1))
                nc.scalar.activation(out=h_sb[:, mc, :], in_=ps, func=AF.Relu,
                                     bias=f1b_sb[:, mc:mc + 1], scale=1.0)

            # ---------------- FF2 + LN2 + output ----------------
            stack2 = single.tile([128, NC_, 512], FP16, tag="stack")
            red2 = red_ps.tile([1, 512], FP32, tag="red")
            for mc in range(NC_):
                wta = ff2_pool.tile([128, DI // 2], FP16, tag="ff2")
                wtb = ff2_pool.tile([128, DI // 2], FP16, tag="ff2")
                nc.sync.dma_start(out=wta, in_=ff2w_b[:][mc, 0])
                nc.scalar.dma_start(out=wtb, in_=ff2w_b[:][mc, 1])
                ps = mm_ps.tile([128, QC], FP32, tag="mm")
                for kc in range(NI):
                    wt = wta if kc < NI // 2 else wtb
                    kcs = kc % (NI // 2)
                    nc.tensor.matmul(ps, wt[:, 128 * kcs:128 * (kcs + 1)],
                                     h_sb[:, kc, :],
                                     start=(kc == 0), stop=(kc == NI - 1))
                nc.vector.scalar_tensor_tensor(
                    out=stack2[:, mc, 0:256], in0=ps, scalar=f2b_sb[:, mc:mc + 1],
                    in1=x_sb[:, mc, :], op0=ALU.add, op1=ALU.add)
                nc.vector.tensor_tensor(
                    out=stack2[:, mc, 256:512], in0=stack2[:, mc, 0:256],
                    in1=stack2[:, mc, 0:256], op=ALU.mult)
                if mc >= 2:
                    nc.tensor.matmul(red2, ones_k, stack2[:, mc - 2, :],
                                     start=(mc == 2), stop=False)
            for mc in (NC_ - 2, NC_ - 1):
                nc.tensor.matmul(red2, ones_k, stack2[:, mc, :],
                                 start=False, stop=(mc == NC_ - 1))

            out_tiles = {}

            def out_tile(mc):
                tt = out_pool.tile([128, QC], FP16, tag="out")
                out_tiles[mc] = tt
                return tt

            layernorm(stack2, red2, out_tile)
            for mc in range(NC_):
                (nc.sync if mc % 2 == 0 else nc.scalar).dma_start(
                    out=out[:][128 * mc:128 * (mc + 1), :], in_=out_tiles[mc])

    nc.compile()
    return nc


def _chunked(a, n):
    # [n*128, w] -> [128, n, w] partition-major
    w = a.shape[1]
    return np.ascontiguousarray(a.reshape(n, 128, w).transpose(1, 0, 2))


def _blocked(wt, nmc, nkc):
    # wt [K, M] (transposed weight) -> [mc, 128, kc*128] lhsT column blocks
    return np.ascontiguousarray(
        wt.reshape(nkc, 128, nmc, 128).transpose(2, 1, 0, 3).reshape(
            nmc, 128, nkc * 128))


def _prep_inputs(z1ss, uss, z0, pos_emb, qkv_w, r_w, r_w_bias, r_r_bias, o_w,
                 o_b, ff1_w, ff1_b, ff2_w, ff2_b):
    f16 = np.float16
    c = np.ascontiguousarray

    wq_t = qkv_w[0:D].T.astype(f16)
    wk_t = qkv_w[D:2 * D].T.astype(f16)
    wv_t = qkv_w[2 * D:3 * D].T.astype(f16)
    shared = dict(
        wq_b=_blocked(wq_t, NC_, NC_), wk_b=_blocked(wk_t, NC_, NC_),
        rw_b=_blocked(r_w.T.astype(f16), NC_, NC_),
        ow_b=_blocked(o_w.T.astype(f16), NC_, NC_),
        ff1w_b=_blocked(ff1_w.T.astype(f16), NI, NC_),
        ff2w_b=_blocked(ff2_w.T.astype(f16), NC_, NI).reshape(
            NC_, 128, 2, DI // 2).transpose(0, 2, 1, 3),
        wv_p=_chunked(wv_t, NC_),
        pos=_chunked(pos_emb[0][:, 768:1024].astype(f16), NC_),
        bwc=_chunked(r_w_bias.reshape(D, 1).astype(np.float32), NC_)[:, :, 0],
        brc=_chunked(r_r_bias.reshape(D, 1).astype(np.float32), NC_)[:, :, 0],
        ob=_chunked(o_b.reshape(D, 1).astype(np.float32), NC_)[:, :, 0],
        f1b=_chunked(ff1_b.reshape(DI, 1).astype(np.float32), NI)[:, :, 0],
        f2b=_chunked(ff2_b.reshape(D, 1).astype(np.float32), NC_)[:, :, 0],
    )
    shared = {k: c(v) for k, v in shared.items()}

    in_maps = []
    for core in range(8):
        b, g = core // 2, core % 2
        q0 = QC * g
        kw0 = q0 + 256
        cat = np.concatenate([z0[b], z1ss[b]], axis=1)
        m = dict(shared)
        m.update(
            xq=_chunked(z1ss[b][:, q0:q0 + QC].astype(f16), NC_),
            xkv=_chunked(cat[:, kw0:kw0 + KW].astype(f16), NC_),
            uq=_chunked(uss[b, 0:D, 512 + q0:512 + q0 + QC].astype(f16), NC_),
            uk=_chunked(uss[b, D:2 * D, kw0:kw0 + KW].astype(f16), NC_),
            uvt=_chunked(uss[b, 2 * D:3 * D, kw0:kw0 + KW].T.astype(f16), 4),
        )
        in_maps.append(m)
    return in_maps


def _get_nc():
    if "nc" not in _CACHE:
        _CACHE["nc"] = build_nc()
    return _CACHE["nc"]


def run(in_maps, trace=False, **kw):
    return run_bass_kernel_spmd(_get_nc(), in_maps, core_ids=list(range(8)),
                                trace=trace, **kw)


def kernel(**inputs):
    inputs = {k: np.asarray(v) for k, v in inputs.items()}
    in_maps = _prep_inputs(**inputs)
    res = run(in_maps)
    bsz, qlen = 4, 512
    full = np.empty((bsz, D, qlen), np.float32)
    for core in range(8):
        b, g = core // 2, core % 2
        full[b][:, QC * g:QC * (g + 1)] = res.results[core]["out"].astype(
            np.float32)
    return full


# revision 39
# speedup vs baseline: 1.0626x; 1.0626x over previous
"""Trainium2 Bass kernel for nn_DEQTransformerLM (Transformer-XL style DEQ layer).

Sharding: local-window attention (window 256, qlen 512, mlen 512) makes the
layer fully token-parallel: 8 cores = 4 batches x 2 query-halves of 256
queries each. Each core runs qkv projections, relative-position attention,
the output projection + layernorm, and the FF block end-to-end for its 256
tokens. No collectives; weights replicated (streamed from HBM as fp16).

rel_shift via DRAM scratch: BD_raw rows [q, 256] written as 385-wide padded
rows (pad = -30000), read back as contiguous [q, 384] rows (stride 384),
which lands BD_raw[i, r] at score column i+1+r and -30000 (additive mask)
elsewhere - mask and shift in one DMA.

Attention softmax pipeline (per 128-query x 384-key quarter):
  AC matmul -> PSUM; BD band added into the same PSUM tile via an
  identity-matmul accumulate; one scalar-engine Exp with accum_out produces
  fp16 probs AND the per-query denominator; [128,1] reciprocal + per-partition
  normalize; fp16 PE transposes; AV accumulated into a shared [128,128] tile.

Phases are arranged so the two hardware DMA queues (sync/scalar, ~120GB/s
each) stay balanced against tensor work; the gpsimd software-DGE queue
carries the BD scratch writes. Q/R projections + BD + attention run as one
software pipeline so the tensor engine never waits on the scratch roundtrip
or the softmax chain.
"""

import numpy as np

import concourse.bacc as bacc
import concourse.mybir as mybir
import concourse.tile as tile
from concourse.bass_utils import run_bass_kernel_spmd
from concourse.masks import make_identity

FP16 = mybir.dt.float16
FP32 = mybir.dt.float32
AF = mybir.ActivationFunctionType
ALU = mybir.AluOpType

D = 1024          # d_model
NH = 16           # heads
DH = 64           # d_head
QC = 256          # queries per core
KW = 512          # key window per core
RW = 256          # relative positions used (cols 768:1024 of r_head_k)
DI = 4096         # d_inner
NC_ = D // 128    # 8 d_model chunks
NI = DI // 128    # 32 d_inner chunks
SCR_ROW = 385     # padded scratch row width
SCR_HEAD = 2 * 128 * SCR_ROW  # scratch block per head
NEG = -30000.0    # additive mask value (fp16-safe)
BW = 384          # in-band score width per 128-query chunk

_CACHE = {}


def build_nc():
    nc = bacc.Bacc("TRN2", target_bir_lowering=False, debug=False)
    P = nc.declare_dram_parameter

    # weights pre-blocked on host as [mc, 128, kc*128] (lhsT column blocks,
    # partition-major) so each block load is one contiguous 2KB-row DMA
    wq_b = P("wq_b", [NC_, 128, D], FP16, isOutput=False)
    wk_b = P("wk_b", [NC_, 128, D], FP16, isOutput=False)
    rw_b = P("rw_b", [NC_, 128, D], FP16, isOutput=False)
    ow_b = P("ow_b", [NC_, 128, D], FP16, isOutput=False)
    ff1w_b = P("ff1w_b", [NI, 128, D], FP16, isOutput=False)
    ff2w_b = P("ff2w_b", [NC_, 2, 128, DI // 2], FP16, isOutput=False)
    # Wv.T row-chunks, split in column halves (head groups 0-7 / 8-15) so the
    # first V-proj half can start as soon as possible
    wv_a = P("wv_a", [128, NC_, D // 2], FP16, isOutput=False)
    wv_b = P("wv_b", [128, NC_, D // 2], FP16, isOutput=False)

    # activations pre-arranged to [128, chunk, cols]
    xq = P("xq", [128, NC_, QC], FP16, isOutput=False)
    xkv = P("xkv", [128, NC_, KW], FP16, isOutput=False)
    uq = P("uq", [128, NC_, QC], FP16, isOutput=False)
    uk = P("uk", [128, NC_, KW], FP16, isOutput=False)
    uvt = P("uvt", [128, 4, D], FP16, isOutput=False)
    pos = P("pos", [128, NC_, RW], FP16, isOutput=False)
    bwc = P("bwc", [128, NC_], FP32, isOutput=False)
    brc = P("brc", [128, NC_], FP32, isOutput=False)
    ob = P("ob", [128, NC_], FP32, isOutput=False)
    f1b = P("f1b", [128, NI], FP32, isOutput=False)
    f2b = P("f2b", [128, NC_], FP32, isOutput=False)

    out = P("out", [D, QC], FP16, isOutput=True)

    scratch = nc.dram_tensor("scratch", [NH * SCR_HEAD], FP16)

    with tile.TileContext(nc) as tc:
        with (
            tc.tile_pool(name="single", bufs=1) as single,
            tc.tile_pool(name="wblk", bufs=6) as wblk_pool,
            tc.tile_pool(name="ff1p", bufs=10) as ff1_pool,
            tc.tile_pool(name="ff2p", bufs=4) as ff2_pool,
            tc.tile_pool(name="bdm", bufs=12) as bdm_pool,
            tc.tile_pool(name="probs", bufs=3) as probs_pool,
            tc.tile_pool(name="pn", bufs=3) as pn_pool,
            tc.tile_pool(name="den", bufs=8) as den_pool,
            tc.tile_pool(name="pt", bufs=3) as pt_pool,
            tc.tile_pool(name="rows", bufs=2) as rows_pool,
            tc.tile_pool(name="outp", bufs=8) as out_pool,
            tc.tile_pool(name="mm_ps", bufs=3, space="PSUM") as mm_ps,
            tc.tile_pool(name="bt_ps", bufs=2, space="PSUM") as bt_ps,
            tc.tile_pool(name="av_ps", bufs=2, space="PSUM") as av_ps_pool,
            tc.tile_pool(name="red_ps", bufs=1, space="PSUM") as red_ps,
        ):
            # ---- loads, ordered by first use. sync + scalar are the two
            # hardware DMA queues; gpsimd (software DGE) carries the BD
            # scratch writes + uvt so the hardware queues stay balanced.
            def load3(name, src, n, w, engine, dtype=FP16, splits=1):
                t = single.tile([128, n, w], dtype, tag=name)
                step = n // splits
                for s in range(splits):
                    engine.dma_start(out=t[:, s * step:(s + 1) * step, :],
                                     in_=src[:][:, s * step:(s + 1) * step, :])
                return t

            xkv_sb = load3("xkv", xkv, NC_, KW, nc.scalar, splits=4)
            uk_sb = load3("uk", uk, NC_, KW, nc.scalar)
            pos_sb = load3("pos", pos, NC_, RW, nc.scalar)
            wvb_sb = load3("wv_b", wv_b, NC_, D // 2, nc.scalar)
            xq_sb = load3("xq", xq, NC_, QC, nc.scalar)
            uq_sb = load3("uq", uq, NC_, QC, nc.scalar)
            uvt_sb = load3("uvt", uvt, 4, D, nc.gpsimd)

            def loadsmall(name, src, w):
                t = single.tile([128, w], FP32, tag=name)
                nc.scalar.dma_start(out=t, in_=src[:])
                return t

            bwc_sb = loadsmall("bwc", bwc, NC_)
            brc_sb = loadsmall("brc", brc, NC_)
            ob_sb = loadsmall("ob", ob, NC_)
            f1b_sb = loadsmall("f1b", f1b, NI)
            f2b_sb = loadsmall("f2b", f2b, NC_)

            ident16 = single.tile([128, 128], FP16, tag="ident16")
            make_identity(nc, ident16)
            ones_k = single.tile([128, 1], FP16, tag="ones_k")
            nc.vector.memset(ones_k, 1.0)
            ones_m = single.tile([1, 128], FP16, tag="ones_m")
            nc.vector.memset(ones_m, 1.0)
            eps_sb = single.tile([1, 1], FP32, tag="eps")
            nc.vector.memset(eps_sb, 1e-5)
            negc_sb = single.tile([128, 1], FP32, tag="negc")
            nc.vector.memset(negc_sb, -12.5)

            pads = single.tile([128, 8, SCR_ROW], FP16, tag="pads")
            nc.gpsimd.memset(pads, NEG)

            k_sb = single.tile([128, NC_, KW], FP16, tag="k_sb")
            vT_sb = single.tile([128, 4, D], FP16, tag="vT_sb")
            rwq_sb = single.tile([128, NC_, QC], FP16, tag="rwq")
            rrq_sb = single.tile([128, NC_, QC], FP16, tag="rrq")
            r_sb = single.tile([128, NC_, RW], FP16, tag="r_sb")
            attn_sb = single.tile([128, NC_, QC], FP16, tag="attn")
            x_sb = single.tile([128, NC_, QC], FP16, tag="x_sb")
            h_sb = single.tile([128, NI, QC], FP16, tag="h_sb")

            def wload(param, mc, engine=None):
                t = wblk_pool.tile([128, D], FP16, tag="wblk")
                (engine or nc.sync).dma_start(out=t, in_=param[:][mc])
                return t

            scr = scratch[:]

            # ---------------- phase A1: K projection ----------
            wva_sb = None
            for mc in range(NC_):
                wt = wload(wk_b, mc)
                if mc == 6:
                    # wv first half slots into sync behind wk0-5, landing
                    # right as K proj drains
                    wva_sb = load3("wv_a", wv_a, NC_, D // 2, nc.sync)
                ps = mm_ps.tile([128, KW], FP32, tag="mm")
                for kc in range(NC_):
                    nc.tensor.matmul(ps, wt[:, 128 * kc:128 * (kc + 1)],
                                     xkv_sb[:, kc, :],
                                     start=(kc == 0), stop=(kc == NC_ - 1))
                nc.vector.tensor_tensor(out=k_sb[:, mc, :], in0=ps,
                                        in1=uk_sb[:, mc, :], op=ALU.add)

            # ---------------- phase A2: V projection (head groups 0-7) ------
            def v_block(jb, dh):
                wvh = wva_sb if dh == 0 else wvb_sb
                ps = mm_ps.tile([128, 512], FP32, tag="mm")
                for kc in range(NC_):
                    nc.tensor.matmul(
                        ps,
                        xkv_sb[:, kc, 128 * jb:128 * (jb + 1)],
                        wvh[:, kc, :],
                        start=(kc == 0), stop=(kc == NC_ - 1))
                nc.vector.tensor_tensor(
                    out=vT_sb[:, jb, 512 * dh:512 * (dh + 1)], in0=ps,
                    in1=uvt_sb[:, jb, 512 * dh:512 * (dh + 1)], op=ALU.add)

            for jb in range(4):
                v_block(jb, 0)

            # ------- phase B: Q/R proj + BD + attention, one pipeline -------
            def q_proj(mc):
                wt = wload(wq_b, mc, nc.sync)
                ps = mm_ps.tile([128, QC], FP32, tag="mm")
                for kc in range(NC_):
                    nc.tensor.matmul(ps, wt[:, 128 * kc:128 * (kc + 1)],
                                     xq_sb[:, kc, :],
                                     start=(kc == 0), stop=(kc == NC_ - 1))
                nc.vector.scalar_tensor_tensor(
                    out=rwq_sb[:, mc, :], in0=ps, scalar=bwc_sb[:, mc:mc + 1],
                    in1=uq_sb[:, mc, :], op0=ALU.add, op1=ALU.add)
                nc.vector.scalar_tensor_tensor(
                    out=rrq_sb[:, mc, :], in0=ps, scalar=brc_sb[:, mc:mc + 1],
                    in1=uq_sb[:, mc, :], op0=ALU.add, op1=ALU.add)

            def r_proj(mc):
                wt = wload(rw_b, mc, nc.sync)
                ps = mm_ps.tile([128, RW], FP32, tag="mm")
                for kc in range(NC_):
                    nc.tensor.matmul(ps, wt[:, 128 * kc:128 * (kc + 1)],
                                     pos_sb[:, kc, :],
                                     start=(kc == 0), stop=(kc == NC_ - 1))
                nc.scalar.copy(r_sb[:, mc, :], ps)

            def bd_half(pi, c):
                # BD matmuls + padded scratch writes, both heads of the pair
                for sub in range(2):
                    h = 2 * pi + sub
                    hsl = slice(64 * sub, 64 * sub + 64)
                    bdp = bt_ps.tile([128, RW], FP32, tag="bt")
                    nc.tensor.matmul(bdp,
                                     rrq_sb[hsl, pi, 128 * c:128 * (c + 1)],
                                     r_sb[hsl, pi, :], start=True, stop=True)
                    slot = (pi % 2) * 4 + 2 * c + sub
                    if sub == 0:
                        nc.vector.tensor_copy(pads[:, slot, 1:257], bdp)
                    else:
                        nc.scalar.copy(pads[:, slot, 1:257], bdp)
                    wbase = h * SCR_HEAD + SCR_ROW * 128 * c
                    nc.gpsimd.dma_start(
                        out=scr[wbase:wbase + 128 * SCR_ROW].rearrange(
                            "(p f) -> p f", f=SCR_ROW),
                        in_=pads[:, slot, :])

            def issue_bdm(pi):
                tiles = []
                for c in range(2):
                    for sub in range(2):
                        h = 2 * pi + sub
                        rbase = h * SCR_HEAD + SCR_ROW * 128 * c
                        t = bdm_pool.tile([128, BW], FP16, tag="bdm")
                        nc.scalar.dma_start(
                            out=t,
                            in_=scr[rbase:rbase + 128 * BW].rearrange(
                                "(p f) -> p f", f=BW))
                        tiles.append(t)
                return tiles

            # Attention: flat 32-quarter software pipeline (quarter = one
            # 128-query chunk x one head). stage1 = score matmuls + exp +
            # normalize; stage2a = fp16 transposes; stage2b = AV. stage2 lags
            # stage1 by two quarters so the in-order tensor queue never waits
            # on the exp/normalize chain.
            def s1(pi, c, sub, bdm):
                hsl = slice(64 * sub, 64 * sub + 64)
                acp = mm_ps.tile([128, BW], FP32, tag="mm")
                nc.tensor.matmul(
                    acp, rwq_sb[hsl, pi, 128 * c:128 * (c + 1)],
                    k_sb[hsl, pi, 128 * c:128 * c + BW],
                    start=True, stop=False)
                nc.tensor.matmul(acp, ident16, bdm, start=False, stop=True)
                probs = probs_pool.tile([128, BW], FP16, tag="probs")
                den = den_pool.tile([128, 1], FP32, tag="den")
                nc.scalar.activation(out=probs, in_=acp, func=AF.Exp,
                                     scale=0.125, bias=negc_sb, accum_out=den)
                rcp = den_pool.tile([128, 1], FP32, tag="rcp")
                nc.vector.reciprocal(out=rcp, in_=den)
                pn = pn_pool.tile([128, BW], FP16, tag="pn")
                nc.vector.tensor_scalar_mul(pn, probs, rcp)
                return pn

            def s2a(qi, pn):
                tp = bt_ps.tile([128, BW], FP16, tag="bt")
                for kbp in range(3):
                    nc.tensor.transpose(tp[:, 128 * kbp:128 * (kbp + 1)],
                                        pn[:, 128 * kbp:128 * (kbp + 1)],
                                        ident16)
                ptc = pt_pool.tile([128, BW], FP16, tag="pt")
                if qi % 2 == 0:
                    nc.scalar.copy(ptc, tp)
                else:
                    nc.vector.tensor_copy(ptc, tp)
                return ptc

            av_tiles = {}

            def s2b(pi, c, sub, ptc):
                h = 2 * pi + sub
                if sub == 0:
                    av = av_ps_pool.tile([128, 128], FP32, tag="av")
                    av_tiles[(pi, c)] = av
                av = av_tiles[(pi, c)]
                for kbp in range(3):
                    nc.tensor.matmul(av[64 * sub:64 * sub + 64, :],
                                     vT_sb[:, c + kbp, 64 * h:64 * h + 64],
                                     ptc[:, 128 * kbp:128 * (kbp + 1)],
                                     start=(kbp == 0), stop=(kbp == 2))
                if sub == 1:
                    nc.scalar.copy(
                        attn_sb[:, pi, 128 * c:128 * (c + 1)],
                        av_tiles.pop((pi, c)))

            # pipeline: per mc emit Q/R/BD for chunk mc and the four attention
            # quarters of pair mc-2 (stage1), with stage2 lagging 2 quarters.
            quarters = [(pi, c, sub)
                        for pi in range(NH // 2)
                        for c in range(2) for sub in range(2)]
            NQ = len(quarters)
            bd_tiles, pns, pts = {}, {}, {}
            t = 0

            def qstep():
                nonlocal t
                if t < NQ:
                    pi, c, sub = quarters[t]
                    pns[t] = s1(pi, c, sub, bd_tiles[pi][2 * c + sub])
                if 0 <= t - 1 < NQ:
                    pts[t - 1] = s2a(t - 1, pns.pop(t - 1))
                if 0 <= t - 2 < NQ:
                    pi2, c2, sub2 = quarters[t - 2]
                    s2b(pi2, c2, sub2, pts.pop(t - 2))
                t += 1

            for mc in range(NC_):
                q_proj(mc)
                if mc >= 1:
                    bd_half(mc - 1, 0)
                r_proj(mc)
                if mc >= 1:
                    bd_half(mc - 1, 1)
                    bd_tiles[mc - 1] = issue_bdm(mc - 1)
                if mc < 4:
                    v_block(mc, 1)
                for _ in range(4):
                    if mc >= 3:
                        qstep()
            bd_half(NC_ - 1, 0)
            bd_half(NC_ - 1, 1)
            bd_tiles[NC_ - 1] = issue_bdm(NC_ - 1)
            while t < NQ + 2:
                qstep()

            # ---------------- O projection + LN1 ----------------
            # LN stats reduce matmuls interleave with the projection loop so
            # the post-loop serial chain is as short as possible.
            stack1 = single.tile([128, NC_, 512], FP16, tag="stack")
            red1 = red_ps.tile([1, 512], FP32, tag="red")
            for mc in range(NC_):
                wt = wload(ow_b, mc, nc.sync if mc % 2 == 0 else nc.scalar)
                ps = mm_ps.tile([128, QC], FP32, tag="mm")
                for kc in range(NC_):
                    nc.tensor.matmul(ps, wt[:, 128 * kc:128 * (kc + 1)],
                                     attn_sb[:, kc, :],
                                     start=(kc == 0), stop=(kc == NC_ - 1))
                nc.vector.scalar_tensor_tensor(
                    out=stack1[:, mc, 0:256], in0=ps, scalar=ob_sb[:, mc:mc + 1],
                    in1=xq_sb[:, mc, :], op0=ALU.add, op1=ALU.add)
                nc.vector.tensor_tensor(
                    out=stack1[:, mc, 256:512], in0=stack1[:, mc, 0:256],
                    in1=stack1[:, mc, 0:256], op=ALU.mult)
                if mc >= 2:
                    nc.tensor.matmul(red1, ones_k, stack1[:, mc - 2, :],
                                     start=(mc == 2), stop=False)
            for mc in (NC_ - 2, NC_ - 1):
                nc.tensor.matmul(red1, ones_k, stack1[:, mc, :],
                                 start=False, stop=(mc == NC_ - 1))

            def layernorm(stack, red, xout):
                mean = rows_pool.tile([1, 256], FP32, tag="mean")
                nc.vector.tensor_scalar_mul(mean, red[0:1, 0:256], 1.0 / D)
                msq = rows_pool.tile([1, 256], FP32, tag="msq")
                nc.vector.tensor_tensor(out=msq, in0=mean, in1=mean,
                                        op=ALU.mult)
                var = rows_pool.tile([1, 256], FP32, tag="var")
                nc.vector.scalar_tensor_tensor(
                    out=var, in0=red[0:1, 256:512], scalar=1.0 / D, in1=msq,
                    op0=ALU.mult, op1=ALU.subtract)
                nc.scalar.activation(out=var, in_=var, func=AF.Sqrt,
                                     bias=eps_sb, scale=1.0)
                rstd = rows_pool.tile([1, 256], FP32, tag="rstd")
                nc.vector.reciprocal_approx_fast(out=rstd, in_=var)
                rw_row = rows_pool.tile([1, 512], FP16, tag="rw_row")
                nc.vector.tensor_copy(rw_row[0:1, 0:256], rstd)
                nc.vector.tensor_tensor(out=rw_row[0:1, 256:512],
                                        in0=mean, in1=rstd, op=ALU.mult)
                bc = mm_ps.tile([128, 512], FP32, tag="mm")
                nc.tensor.matmul(bc, ones_m, rw_row, start=True, stop=True)
                bc_sb = rows_pool.tile([128, 512], FP16, tag="bc_sb")
                nc.vector.tensor_copy(bc_sb, bc)
                for mc in range(NC_):
                    xo = xout(mc)
                    eng = nc.vector if mc < 6 else nc.gpsimd
                    eng.tensor_tensor(out=xo, in0=stack[:, mc, 0:256],
                                      in1=bc_sb[:, 0:256], op=ALU.mult)
                    eng.tensor_tensor(out=xo, in0=xo,
                                      in1=bc_sb[:, 256:512], op=ALU.subtract)

            layernorm(stack1, red1, lambda mc: x_sb[:, mc, :])

            # ---------------- FF1 ----------------
            for mc in range(NI):
                wt = ff1_pool.tile([128, D], FP16, tag="ff1")
                (nc.sync if mc % 2 == 0 else nc.scalar).dma_start(
                    out=wt, in_=ff1w_b[:][mc])
                ps = mm_ps.tile([128, QC], FP32, tag="mm")
                for kc in range(NC_):
                    nc.tensor.matmul(ps, wt[:, 128 * kc:128 * (kc + 1)],
                                     x_sb[:, kc, :],
                                     start=(kc == 0), stop=(kc == NC_ - 1))
                nc.scalar.activation(out=h_sb[:, mc, :], in_=ps, func=AF.Relu,
                                     bias=f1b_sb[:, mc:mc + 1], scale=1.0)

            # ---------------- FF2 + LN2 + output ----------------
            stack2 = single.tile([128, NC_, 512], FP16, tag="stack")
            red2 = red_ps.tile([1, 512], FP32, tag="red")
            for mc in range(NC_):
                wta = ff2_pool.tile([128, DI // 2], FP16, tag="ff2")
                wtb = ff2_pool.tile([128, DI // 2], FP16, tag="ff2")
                nc.sync.dma_start(out=wta, in_=ff2w_b[:][mc, 0])
                nc.scalar.dma_start(out=wtb, in_=ff2w_b[:][mc, 1])
                ps = mm_ps.tile([128, QC], FP32, tag="mm")
                for kc in range(NI):
                    wt = wta if kc < NI // 2 else wtb
                    kcs = kc % (NI // 2)
                    nc.tensor.matmul(ps, wt[:, 128 * kcs:128 * (kcs + 1)],
                                     h_sb[:, kc, :],
                                     start=(kc == 0), stop=(kc == NI - 1))
                nc.vector.scalar_tensor_tensor(
                    out=stack2[:, mc, 0:256], in0=ps, scalar=f2b_sb[:, mc:mc + 1],
                    in1=x_sb[:, mc, :], op0=ALU.add, op1=ALU.add)
                nc.vector.tensor_tensor(
                    out=stack2[:, mc, 256:512], in0=stack2[:, mc, 0:256],
                    in1=stack2[:, mc, 0:256], op=ALU.mult)
                if mc >= 2:
                    nc.tensor.matmul(red2, ones_k, stack2[:, mc - 2, :],
                                     start=(mc == 2), stop=False)
            for mc in (NC_ - 2, NC_ - 1):
                nc.tensor.matmul(red2, ones_k, stack2[:, mc, :],
                                 start=False, stop=(mc == NC_ - 1))

            out_tiles = {}

            def out_tile(mc):
                tt = out_pool.tile([128, QC], FP16, tag="out")
                out_tiles[mc] = tt
                return tt

            layernorm(stack2, red2, out_tile)
            for mc in range(NC_):
                (nc.sync if mc % 2 == 0 else nc.scalar).dma_start(
                    out=out[:][128 * mc:128 * (mc + 1), :], in_=out_tiles[mc])

    nc.compile()
    return nc


def _chunked(a, n):
    # [n*128, w] -> [128, n, w] partition-major
    w = a.shape[1]
    return np.ascontiguousarray(a.reshape(n, 128, w).transpose(1, 0, 2))


def _blocked(wt, nmc, nkc):
    # wt [K, M] (transposed weight) -> [mc, 128, kc*128] lhsT column blocks
    return np.ascontiguousarray(
        wt.reshape(nkc, 128, nmc, 128).transpose(2, 1, 0, 3).reshape(
            nmc, 128, nkc * 128))


def _prep_inputs(z1ss, uss, z0, pos_emb, qkv_w, r_w, r_w_bias, r_r_bias, o_w,
                 o_b, ff1_w, ff1_b, ff2_w, ff2_b):
    f16 = np.float16
    c = np.ascontiguousarray

    wq_t = qkv_w[0:D].T.astype(f16)
    wk_t = qkv_w[D:2 * D].T.astype(f16)
    wv_t = qkv_w[2 * D:3 * D].T.astype(f16)
    shared = dict(
        wq_b=_blocked(wq_t, NC_, NC_), wk_b=_blocked(wk_t, NC_, NC_),
        rw_b=_blocked(r_w.T.astype(f16), NC_, NC_),
        ow_b=_blocked(o_w.T.astype(f16), NC_, NC_),
        ff1w_b=_blocked(ff1_w.T.astype(f16), NI, NC_),
        ff2w_b=_blocked(ff2_w.T.astype(f16), NC_, NI).reshape(
            NC_, 128, 2, DI // 2).transpose(0, 2, 1, 3),
        wv_a=_chunked(wv_t, NC_)[:, :, 0:512],
        wv_b=_chunked(wv_t, NC_)[:, :, 512:1024],
        pos=_chunked(pos_emb[0][:, 768:1024].astype(f16), NC_),
        bwc=_chunked(r_w_bias.reshape(D, 1).astype(np.float32), NC_)[:, :, 0],
        brc=_chunked(r_r_bias.reshape(D, 1).astype(np.float32), NC_)[:, :, 0],
        ob=_chunked(o_b.reshape(D, 1).astype(np.float32), NC_)[:, :, 0],
        f1b=_chunked(ff1_b.reshape(DI, 1).astype(np.float32), NI)[:, :, 0],
        f2b=_chunked(ff2_b.reshape(D, 1).astype(np.float32), NC_)[:, :, 0],
    )
    shared = {k: c(v) for k, v in shared.items()}

    in_maps = []
    for core in range(8):
        b, g = core // 2, core % 2
        q0 = QC * g
        kw0 = q0 + 256
        cat = np.concatenate([z0[b], z1ss[b]], axis=1)
        m = dict(shared)
        m.update(
            xq=_chunked(z1ss[b][:, q0:q0 + QC].astype(f16), NC_),
            xkv=_chunked(cat[:, kw0:kw0 + KW].astype(f16), NC_),
            uq=_chunked(uss[b, 0:D, 512 + q0:512 + q0 + QC].astype(f16), NC_),
            uk=_chunked(uss[b, D:2 * D, kw0:kw0 + KW].astype(f16), NC_),
            uvt=_chunked(uss[b, 2 * D:3 * D, kw0:kw0 + KW].T.astype(f16), 4),
        )
        in_maps.append(m)
    return in_maps


def _get_nc():
    if "nc" not in _CACHE:
        _CACHE["nc"] = build_nc()
    return _CACHE["nc"]


def run(in_maps, trace=False, **kw):
    return run_bass_kernel_spmd(_get_nc(), in_maps, core_ids=list(range(8)),
                                trace=trace, **kw)


def kernel(**inputs):
    inputs = {k: np.asarray(v) for k, v in inputs.items()}
    in_maps = _prep_inputs(**inputs)
    res = run(in_maps)
    bsz, qlen = 4, 512
    full = np.empty((bsz, D, qlen), np.float32)
    for core in range(8):
        b, g = core // 2, core % 2
        full[b][:, QC * g:QC * (g + 1)] = res.results[core]["out"].astype(
            np.float32)
    return full
